# revision 23
# baseline (speedup 1.0000x reference)
"""CombinePatches (3D col2im fold + overlap-count normalize) on 8 TRN2 NeuronCores.

Decomposition (validated numerically against the reference):
  out[b, 2q+kd, 2s+kh, 2u+kw, c] (+)= patches[b, q, s, u, kd, kh, kw, c], then
  out /= cnt, cnt = cd(d)*ch(h)*cw(w) separable overlap counts.

Sharding: 8 cores = B(2) x D-chunks(4). Each core computes 16 output d-rows from
9 od-slices of patches (1 halo slice, zero-padded at global edges by the host).

Per core, per output row d (r=d%2, q=d//2):
  - DVE w-fold: T[s, j, w, c] = A[s, floor(w/2), j, ...] + A[s, floor(w/2)-1, ...]
    done for A = slice q (kd=r) and B = slice q-1 (kd=r+2), with the ow dim
    pre-split into two halves on partitions (p = uhalf*64 + s) so each DVE op
    uses all 128 lanes.
  - TensorE h-fold: O[h, (w,c)] = sum_j Mh_j^T @ T_j accumulated in PSUM over
    (j x {A,B} x {w-half}) = 16 float32r matmuls; 0.25*rh(h) baked into Mh
    (0.25 = interior rd * interior rw).
  - ScalarE eviction: PSUM -> SBUF copy, then DMA store on the scalar ring.
Host fixes the global d-edge rows and w-edge columns by x2 after gather.
"""
import sys

for _p in ("/opt/trn_rl_repo", "/opt/trn_rl_repo/pypackages"):
    if _p not in sys.path:
        sys.path.insert(0, _p)

from contextlib import ExitStack

import numpy as np

import concourse.bass as bass
import concourse.tile as tile
from concourse import bacc, mybir
from concourse import bass_utils

B, D, H, W, C = 2, 64, 128, 128, 4
od, oh, ow = 31, 63, 63
NS = 9              # od-slices per core (incl 1 halo)
RPC = 16            # output d-rows per core
MM_DT = mybir.dt.bfloat16
import ml_dtypes

BF16 = ml_dtypes.bfloat16

# per-partition folded free width of a slice with nkd kd-planes:
# [x=32][kd=nkd][j=4][t=2][c=4]; DRAM stages 2 vpair copies of it.
def _fwh(nkd):
    return 32 * nkd * 32


FULL2, HALF2 = 2 * _fwh(4), 2 * _fwh(2)   # DRAM elems/partition per slice
PP_TOTAL = 128 * (2 * HALF2 + 7 * FULL2)

_cache = {}


def _build():
    nc = bacc.Bacc(
        "TRN2",
        target_bir_lowering=False,
        debug=False,
        enable_asserts=False,
        num_devices=8,
    )
    # flat pp: [half-slice k=0 (kd 2,3 only)] + [7 full slices] + [half k=8 (kd 0,1)]
    pp_d = nc.dram_tensor(
        "pp", [PP_TOTAL], MM_DT, kind="ExternalInput"
    ).ap()
    wm_d = nc.dram_tensor("wm", [128, 1024], MM_DT, kind="ExternalInput").ap()
    out_d = nc.dram_tensor(
        "out", [RPC, H, W, C], MM_DT, kind="ExternalOutput"
    ).ap()

    with ExitStack() as ctx:
        tc = ctx.enter_context(tile.TileContext(nc))
        const_pool = ctx.enter_context(tc.tile_pool(name="const", bufs=1))
        # bufs=6: enough slot slack that compute hiccups don't stall the load
        # stream, but not so many outstanding DMAs that packets slow down
        # (9 outstanding loads measurably degraded early HBM throughput).
        slice_pool = ctx.enter_context(tc.tile_pool(name="slice", bufs=6))
        t_pool = ctx.enter_context(tc.tile_pool(name="tt", bufs=6))
        ev_pool = ctx.enter_context(tc.tile_pool(name="ev", bufs=3))
        psum_pool = ctx.enter_context(tc.tile_pool(name="ps", bufs=3, space="PSUM"))

        # constants go on the scalar-engine HWDGE ring so the sync ring is
        # purely slice loads (HWDGE rings are FIFO per issuing engine).
        wm_sb = const_pool.tile([128, 1024], MM_DT)
        nc.scalar.dma_start(wm_sb[:], wm_d[:])

        def slice_region(k):
            """(flat offset, folded free width, n_kd, kd_base) of slice k."""
            if k == 0:
                return 0, _fwh(2), 2, 2
            if k == NS - 1:
                return 128 * (HALF2 + 7 * FULL2), _fwh(2), 2, 0
            return 128 * (HALF2 + (k - 1) * FULL2), _fwh(4), 4, 0

        tiles = {}
        for k in range(NS):
            off, fwh, nkd, kdb = slice_region(k)
            t = slice_pool.tile([128, fwh], MM_DT, tag="slice")
            region = pp_d[off : off + 128 * 2 * fwh].rearrange(
                "(p v f) -> p v f", v=2, f=fwh
            )
            # kw-fold happens inside the DMA: plain HWDGE load of the kw 0,1
            # stream, then SWDGE accumulate loads of the kw 2,3 stream onto
            # the same tile (CCE inline adder) -- zero engine cost and the
            # SBUF slice tile is half the DRAM footprint. Accum DMAs larger
            # than ~512KB fault the exec unit, so chunk at 2048 elems/part.
            nc.sync.dma_start(t[:], region[:, 0, :])
            for h0 in range(0, fwh, 2048):
                h1 = min(h0 + 2048, fwh)
                nc.gpsimd.dma_start(
                    t[:, h0:h1],
                    region[:, 1, h0:h1],
                    accum_op=mybir.AluOpType.add,
                )
            tiles[k] = (t, nkd, kdb)
            if k == 0:
                continue
            # one PSUM tile (2 banks), one eviction, one store per slice
            # (= 2 output rows): fewer DMAs and semaphores shrink both the
            # serialized scalar work and the fixed end-of-NEFF sem-drain.
            ps = psum_pool.tile([128, 1024], mybir.dt.float32, tag="ps")
            for rr in range(2):
                # single DVE op per row: kd-fold of the two slices' folded
                # planes, written j-major for the matmul rhs blocks.
                TA = t_pool.tile([128, 1024], MM_DT, tag="T")
                ta, a_nkd, a_kdb = tiles[k]
                tb, b_nkd, b_kdb = tiles[k - 1]
                va = ta[:].rearrange(
                    "p (x kd j t c) -> p x kd j t c", x=32, kd=a_nkd, j=4, t=2, c=4
                )
                vb = tb[:].rearrange(
                    "p (x kd j t c) -> p x kd j t c", x=32, kd=b_nkd, j=4, t=2, c=4
                )
                pa = va[:, :, rr - a_kdb].rearrange("p m j t c -> p j m t c")
                pb = vb[:, :, rr + 2 - b_kdb].rearrange("p m j t c -> p j m t c")
                To = TA[:].rearrange("p (j m t c) -> p j m t c", j=4, m=32, t=2, c=4)
                nc.vector.tensor_add(To, pa, pb)
                for half in range(2):
                    outseg = ps[:, rr * 512 + half * 256 : rr * 512 + (half + 1) * 256]
                    for j in range(4):
                        # K=128 with zero-padded block-diagonal weights keeps
                        # every matmul at tile_position (0,0): mixing PE tile
                        # positions in one NEFF hangs at runtime.
                        lhsT = wm_sb[:, 512 * half + j * 128 : 512 * half + (j + 1) * 128]
                        rhs = TA[:, j * 256 : (j + 1) * 256]
                        nc.tensor.matmul(
                            outseg, lhsT, rhs, start=(j == 0), stop=(j == 3)
                        )
            # evict on ScalarE: evictions wait on matmuls, and in the DVE
            # FIFO they would delay later w-folds, which gate slice loads
            # via slot release. rw's interior 0.5 is folded into wm; the
            # host rescales the 4 global w-edge columns.
            ev = ev_pool.tile([128, 1024], MM_DT, tag="ev")
            nc.scalar.copy(ev[:], ps[:])
            # stores on the scalar ring: a store waiting on eviction must
            # not head-of-line-block the next slice load on the sync ring
            d0 = 2 * (k - 1)
            nc.scalar.dma_start(
                out_d[d0 : d0 + 2].rearrange("d h w c -> h d (w c)"),
                ev[:].rearrange("p (d f) -> p d f", d=2),
            )
    nc.compile()
    return nc


def _host_tables():
    rh = np.where(
        (np.arange(H) < 2) | (np.arange(H) >= H - 2), 1.0, 0.5
    ).astype(np.float32)
    # [half*64+s, whalf*512 + j*128 + h], block-diagonal in (half, whalf).
    # 0.25 = interior rd (0.5) * interior rw (0.5); host rescales d/w edges.
    wm = np.zeros((128, 1024), np.float32)
    s_idx = np.arange(oh)
    for j in range(4):
        h = 2 * s_idx + j
        wm[s_idx, j * 128 + h] = 0.25 * rh[h]
        wm[64 + s_idx, 512 + j * 128 + h] = 0.25 * rh[h]
    return wm.astype(BF16)


def _shard_inputs(patches):
    """Build per-core flat patch blocks. Per slice the layout is
    [p=(uhalf,s)][vpair][x=32][kd][j][t][c] where vpair 0 = kw{0,1} at
    u-slots 1:33 and vpair 1 = kw{2,3} at u-slots 0:32, so the device's
    accumulate-DMA of vpair 1 onto vpair 0 performs the kw-fold."""
    P5 = np.ascontiguousarray(patches).reshape(B, od, oh, ow, 256).astype(BF16)
    # q-slot k = q+1 for q in [-1, 32); u-slot x = u+1 for u in [-1, 65)
    Pu = np.zeros((B, od + 2, 64, 66, 4, 4, 4, 4), BF16)
    Pu.reshape(B, od + 2, 64, 66, 256)[:, 1 : od + 1, 0:oh, 1 : ow + 1, :] = P5
    pps = []
    for core in range(8):
        b, kc = core // 4, core % 4
        s0 = 8 * kc  # = qbase + 1
        parts = []
        for k in range(NS):
            if k == 0:
                nkd, kdb = 2, 2
            elif k == NS - 1:
                nkd, kdb = 2, 0
            else:
                nkd, kdb = 4, 0
            Q = Pu[b, s0 + k]  # [s=64, u=66, kd, j, v, c]
            blk = np.empty((2, 64, 2, 32, nkd, 4, 2, 4), BF16)
            for uh in range(2):
                for vp in range(2):
                    us = 32 * uh + (1 - vp)
                    blk[uh, :, vp] = Q[
                        :, us : us + 32, kdb : kdb + nkd, :, 2 * vp : 2 * vp + 2, :
                    ]
            parts.append(blk.reshape(-1))
        pps.append(np.concatenate(parts))
    return pps


def _run(patches, trace=False):
    if "nc" not in _cache:
        _cache["nc"] = _build()
        _cache["tables"] = _host_tables()
    nc = _cache["nc"]
    wm = _cache["tables"]
    pps = _shard_inputs(np.asarray(patches, dtype=np.float32))
    in_maps = [{"pp": pps[core], "wm": wm} for core in range(8)]
    res = bass_utils.run_bass_kernel_spmd(
        nc, in_maps, core_ids=list(range(8)), trace=trace
    )
    out = np.zeros((B, D, H, W, C), np.float32)
    for core in range(8):
        b, kc = core // 4, core % 4
        out[b, RPC * kc : RPC * (kc + 1)] = np.asarray(
            res.results[core]["out"]
        ).astype(np.float32)
    out[:, [0, 1, D - 2, D - 1]] *= 2.0
    out[:, :, :, [0, 1, W - 2, W - 1], :] *= 2.0
    return out, res


def kernel(patches, inputs):
    out, _ = _run(patches)
    return out



# revision 26
# speedup vs baseline: 1.1824x; 1.1824x over previous
"""CombinePatches (3D col2im fold + overlap-count normalize) on 8 TRN2 NeuronCores.

Decomposition (validated numerically against the reference):
  out[b, 2q+kd, 2s+kh, 2u+kw, c] (+)= patches[b, q, s, u, kd, kh, kw, c], then
  out /= cnt, cnt = cd(d)*ch(h)*cw(w) separable overlap counts.

Sharding: 8 cores = B(2) x D-chunks(4). Each core computes 16 output d-rows from
9 od-slices of patches (1 halo slice, zero-padded at global edges by the host).

Per core, per output row d (r=d%2, q=d//2):
  - DVE w-fold: T[s, j, w, c] = A[s, floor(w/2), j, ...] + A[s, floor(w/2)-1, ...]
    done for A = slice q (kd=r) and B = slice q-1 (kd=r+2), with the ow dim
    pre-split into two halves on partitions (p = uhalf*64 + s) so each DVE op
    uses all 128 lanes.
  - TensorE h-fold: O[h, (w,c)] = sum_j Mh_j^T @ T_j accumulated in PSUM over
    (j x {A,B} x {w-half}) = 16 float32r matmuls; 0.25*rh(h) baked into Mh
    (0.25 = interior rd * interior rw).
  - ScalarE eviction: PSUM -> SBUF copy, then DMA store on the scalar ring.
Host fixes the global d-edge rows and w-edge columns by x2 after gather.
"""
import sys

for _p in ("/opt/trn_rl_repo", "/opt/trn_rl_repo/pypackages"):
    if _p not in sys.path:
        sys.path.insert(0, _p)

from contextlib import ExitStack

import numpy as np

import concourse.bass as bass
import concourse.tile as tile
from concourse import bacc, mybir
from concourse import bass_utils

B, D, H, W, C = 2, 64, 128, 128, 4
od, oh, ow = 31, 63, 63
NS = 9              # od-slices per core (incl 1 halo)
RPC = 16            # output d-rows per core
MM_DT = mybir.dt.bfloat16
import ml_dtypes

BF16 = ml_dtypes.bfloat16

# per-partition free width of a slice with nkd kd-planes:
# [kd=nkd][vpair=2][x=32][j=4][t=2][c=4] -- each (kd, vpair) plane is a
# contiguous 1024-elem run, so every DVE operand is stride-1.
def _fw(nkd):
    return nkd * 2 * 1024


FULL2, HALF2 = _fw(4), _fw(2)   # DRAM elems/partition per slice
PP_TOTAL = 128 * (2 * HALF2 + 7 * FULL2)

_cache = {}


def _build():
    nc = bacc.Bacc(
        "TRN2",
        target_bir_lowering=False,
        debug=False,
        enable_asserts=False,
        num_devices=8,
    )
    # flat pp: [half-slice k=0 (kd 2,3 only)] + [7 full slices] + [half k=8 (kd 0,1)]
    pp_d = nc.dram_tensor(
        "pp", [PP_TOTAL], MM_DT, kind="ExternalInput"
    ).ap()
    wm_d = nc.dram_tensor("wm", [128, 1024], MM_DT, kind="ExternalInput").ap()
    out_d = nc.dram_tensor(
        "out", [RPC, H, W, C], MM_DT, kind="ExternalOutput"
    ).ap()

    with ExitStack() as ctx:
        tc = ctx.enter_context(tile.TileContext(nc))
        const_pool = ctx.enter_context(tc.tile_pool(name="const", bufs=1))
        # bufs=6: enough slot slack that compute hiccups don't stall the load
        # stream, but not so many outstanding DMAs that packets slow down
        # (9 outstanding loads measurably degraded early HBM throughput).
        slice_pool = ctx.enter_context(tc.tile_pool(name="slice", bufs=6))
        t_pool = ctx.enter_context(tc.tile_pool(name="tt", bufs=6))
        ev_pool = ctx.enter_context(tc.tile_pool(name="ev", bufs=3))
        psum_pool = ctx.enter_context(tc.tile_pool(name="ps", bufs=3, space="PSUM"))

        # constants go on the scalar-engine HWDGE ring so the sync ring is
        # purely slice loads (HWDGE rings are FIFO per issuing engine).
        wm_sb = const_pool.tile([128, 1024], MM_DT)
        nc.scalar.dma_start(wm_sb[:], wm_d[:])

        def slice_region(k):
            """(flat offset, free width, n_kd, kd_base) of slice k."""
            if k == 0:
                return 0, HALF2, 2, 2
            if k == NS - 1:
                return 128 * (HALF2 + 7 * FULL2), HALF2, 2, 0
            return 128 * (HALF2 + (k - 1) * FULL2), FULL2, 4, 0

        tiles = {}
        for k in range(NS):
            off, fw, nkd, kdb = slice_region(k)
            t = slice_pool.tile([128, fw], MM_DT, tag="slice")
            src = pp_d[off : off + 128 * fw].rearrange("(p f) -> p f", f=fw)
            nc.sync.dma_start(t[:], src)
            tiles[k] = (t, nkd, kdb)
            if k == 0:
                continue
            # one PSUM tile (2 banks), one eviction, one store per slice
            # (= 2 output rows): fewer DMAs and semaphores shrink both the
            # serialized scalar work and the fixed end-of-NEFF sem-drain.
            ps = psum_pool.tile([128, 1024], mybir.dt.float32, tag="ps")
            for rr in range(2):
                # 3 DVE adds per row, every operand a stride-1 [p, 1024] run:
                # kw-fold of slice k (kd=rr), kw-fold of slice k-1 (kd=rr+2),
                # then the kd-fold combine. Contiguity keeps DVE SBUF-port
                # traffic minimal -- strided 8-elem runs waste half of every
                # 32B line and that bank pressure slows DMA/PE under load.
                TA = t_pool.tile([128, 1024], MM_DT, tag="T")
                TB = t_pool.tile([128, 1024], MM_DT, tag="T")
                for T, (tk, t_nkd, t_kdb), kd in (
                    (TA, tiles[k], rr),
                    (TB, tiles[k - 1], rr + 2),
                ):
                    v = tk[:].rearrange(
                        "p (kd v f) -> p kd v f", kd=t_nkd, v=2, f=1024
                    )
                    ki = kd - t_kdb
                    nc.vector.tensor_add(T[:], v[:, ki, 0, :], v[:, ki, 1, :])
                nc.vector.tensor_add(TA[:], TA[:], TB[:])
                # T layout is (m, j, t, c): the matmul rhs j-blocks become
                # strided views (m:32-stride, 8 contig) -- PE streams by AP.
                rv = TA[:].rearrange("p (m j t c) -> p j m t c", m=32, j=4, t=2, c=4)
                for half in range(2):
                    outseg = ps[:, rr * 512 + half * 256 : rr * 512 + (half + 1) * 256]
                    for j in range(4):
                        # K=128 with zero-padded block-diagonal weights keeps
                        # every matmul at tile_position (0,0): mixing PE tile
                        # positions in one NEFF hangs at runtime.
                        lhsT = wm_sb[:, 512 * half + j * 128 : 512 * half + (j + 1) * 128]
                        nc.tensor.matmul(
                            outseg, lhsT, rv[:, j], start=(j == 0), stop=(j == 3)
                        )
            # evict on ScalarE: evictions wait on matmuls, and in the DVE
            # FIFO they would delay later w-folds, which gate slice loads
            # via slot release. rw's interior 0.5 is folded into wm; the
            # host rescales the 4 global w-edge columns.
            ev = ev_pool.tile([128, 1024], MM_DT, tag="ev")
            nc.scalar.copy(ev[:], ps[:])
            # stores on the scalar ring: a store waiting on eviction must
            # not head-of-line-block the next slice load on the sync ring
            d0 = 2 * (k - 1)
            nc.scalar.dma_start(
                out_d[d0 : d0 + 2].rearrange("d h w c -> h d (w c)"),
                ev[:].rearrange("p (d f) -> p d f", d=2),
            )
    nc.compile()
    return nc


def _host_tables():
    rh = np.where(
        (np.arange(H) < 2) | (np.arange(H) >= H - 2), 1.0, 0.5
    ).astype(np.float32)
    # [half*64+s, whalf*512 + j*128 + h], block-diagonal in (half, whalf).
    # 0.25 = interior rd (0.5) * interior rw (0.5); host rescales d/w edges.
    wm = np.zeros((128, 1024), np.float32)
    s_idx = np.arange(oh)
    for j in range(4):
        h = 2 * s_idx + j
        wm[s_idx, j * 128 + h] = 0.25 * rh[h]
        wm[64 + s_idx, 512 + j * 128 + h] = 0.25 * rh[h]
    return wm.astype(BF16)


def _shard_inputs(patches):
    """Build per-core flat patch blocks. Per slice the layout is
    [p=(uhalf,s)][kd][vpair][x=32][j][t][c] where vpair 0 = kw{0,1} at
    u-slots 1:33 and vpair 1 = kw{2,3} at u-slots 0:32; each (kd, vpair)
    plane is a contiguous 1024-elem DVE operand."""
    P5 = np.ascontiguousarray(patches).reshape(B, od, oh, ow, 256).astype(BF16)
    # q-slot k = q+1 for q in [-1, 32); u-slot x = u+1 for u in [-1, 65)
    Pu = np.zeros((B, od + 2, 64, 66, 4, 4, 4, 4), BF16)
    Pu.reshape(B, od + 2, 64, 66, 256)[:, 1 : od + 1, 0:oh, 1 : ow + 1, :] = P5
    pps = []
    for core in range(8):
        b, kc = core // 4, core % 4
        s0 = 8 * kc  # = qbase + 1
        parts = []
        for k in range(NS):
            if k == 0:
                nkd, kdb = 2, 2
            elif k == NS - 1:
                nkd, kdb = 2, 0
            else:
                nkd, kdb = 4, 0
            Q = Pu[b, s0 + k]  # [s=64, u=66, kd, j, v, c]
            blk = np.empty((2, 64, nkd, 2, 32, 4, 2, 4), BF16)
            for uh in range(2):
                for vp in range(2):
                    us = 32 * uh + (1 - vp)
                    # [s, x, kd, j, t, c] -> [s, kd, x, j, t, c]
                    blk[uh, :, :, vp] = np.moveaxis(
                        Q[:, us : us + 32, kdb : kdb + nkd, :, 2 * vp : 2 * vp + 2, :],
                        1,
                        2,
                    )
            parts.append(blk.reshape(-1))
        pps.append(np.concatenate(parts))
    return pps


def _run(patches, trace=False):
    if "nc" not in _cache:
        _cache["nc"] = _build()
        _cache["tables"] = _host_tables()
    nc = _cache["nc"]
    wm = _cache["tables"]
    pps = _shard_inputs(np.asarray(patches, dtype=np.float32))
    in_maps = [{"pp": pps[core], "wm": wm} for core in range(8)]
    res = bass_utils.run_bass_kernel_spmd(
        nc, in_maps, core_ids=list(range(8)), trace=trace
    )
    out = np.zeros((B, D, H, W, C), np.float32)
    for core in range(8):
        b, kc = core // 4, core % 4
        out[b, RPC * kc : RPC * (kc + 1)] = np.asarray(
            res.results[core]["out"]
        ).astype(np.float32)
    out[:, [0, 1, D - 2, D - 1]] *= 2.0
    out[:, :, :, [0, 1, W - 2, W - 1], :] *= 2.0
    return out, res


def kernel(patches, inputs):
    out, _ = _run(patches)
    return out



# revision 31
# speedup vs baseline: 1.1905x; 1.0069x over previous
"""CombinePatches (3D col2im fold + overlap-count normalize) on 8 TRN2 NeuronCores.

Decomposition (validated numerically against the reference):
  out[b, 2q+kd, 2s+kh, 2u+kw, c] (+)= patches[b, q, s, u, kd, kh, kw, c], then
  out /= cnt, cnt = cd(d)*ch(h)*cw(w) separable overlap counts.

Sharding: 8 cores = B(2) x D-chunks(4). Each core computes 16 output d-rows from
9 od-slices of patches (1 halo slice, zero-padded at global edges by the host).

Per core, per output row d (r=d%2, q=d//2):
  - DVE w-fold: T[s, j, w, c] = A[s, floor(w/2), j, ...] + A[s, floor(w/2)-1, ...]
    done for A = slice q (kd=r) and B = slice q-1 (kd=r+2), with the ow dim
    pre-split into two halves on partitions (p = uhalf*64 + s) so each DVE op
    uses all 128 lanes.
  - TensorE h-fold: O[h, (w,c)] = sum_j Mh_j^T @ T_j accumulated in PSUM over
    (j x {A,B} x {w-half}) = 16 float32r matmuls; 0.25*rh(h) baked into Mh
    (0.25 = interior rd * interior rw).
  - ScalarE eviction: PSUM -> SBUF copy, then DMA store on the scalar ring.
Host fixes the global d-edge rows and w-edge columns by x2 after gather.
"""
import sys

for _p in ("/opt/trn_rl_repo", "/opt/trn_rl_repo/pypackages"):
    if _p not in sys.path:
        sys.path.insert(0, _p)

from contextlib import ExitStack

import numpy as np

import concourse.bass as bass
import concourse.tile as tile
from concourse import bacc, mybir
from concourse import bass_utils

B, D, H, W, C = 2, 64, 128, 128, 4
od, oh, ow = 31, 63, 63
NS = 9              # od-slices per core (incl 1 halo)
RPC = 16            # output d-rows per core
MM_DT = mybir.dt.bfloat16
import ml_dtypes

BF16 = ml_dtypes.bfloat16

# per-partition free width of a slice with nkd kd-planes:
# [vpair=2][kd=nkd][j=4][x=32][t=2][c=4] -- vpair outermost, so the whole
# kw-fold of a slice is ONE fully contiguous DVE add (vp0 half + vp1 half),
# and each folded kd-plane is a contiguous j-major 1024-elem matmul rhs.
def _fw(nkd):
    return 2 * nkd * 1024


FULL2, HALF2 = _fw(4), _fw(2)   # DRAM elems/partition per slice
PP_TOTAL = 128 * (2 * HALF2 + 7 * FULL2)

_cache = {}


def _build():
    nc = bacc.Bacc(
        "TRN2",
        target_bir_lowering=False,
        debug=False,
        enable_asserts=False,
        num_devices=8,
    )
    # flat pp: [half-slice k=0 (kd 2,3 only)] + [7 full slices] + [half k=8 (kd 0,1)]
    pp_d = nc.dram_tensor(
        "pp", [PP_TOTAL], MM_DT, kind="ExternalInput"
    ).ap()
    wm_d = nc.dram_tensor("wm", [128, 1024], MM_DT, kind="ExternalInput").ap()
    out_d = nc.dram_tensor(
        "out", [RPC, H, W, C], MM_DT, kind="ExternalOutput"
    ).ap()

    with ExitStack() as ctx:
        tc = ctx.enter_context(tile.TileContext(nc))
        const_pool = ctx.enter_context(tc.tile_pool(name="const", bufs=1))
        # staged slice tiles have exactly one reader (the mega-fold), so
        # slots recycle immediately and a few bufs keep the DMA stream fed
        # without piling up outstanding DMAs (9 outstanding loads measurably
        # degraded early HBM throughput).
        slice_pool = ctx.enter_context(tc.tile_pool(name="slice", bufs=4))
        f_pool = ctx.enter_context(tc.tile_pool(name="fold", bufs=3))
        t_pool = ctx.enter_context(tc.tile_pool(name="tt", bufs=6))
        ev_pool = ctx.enter_context(tc.tile_pool(name="ev", bufs=3))
        psum_pool = ctx.enter_context(tc.tile_pool(name="ps", bufs=3, space="PSUM"))

        # constants go on the scalar-engine HWDGE ring so the sync ring is
        # purely slice loads (HWDGE rings are FIFO per issuing engine).
        wm_sb = const_pool.tile([128, 1024], MM_DT)
        nc.scalar.dma_start(wm_sb[:], wm_d[:])

        def slice_region(k):
            """(flat offset, free width, n_kd, kd_base) of slice k."""
            if k == 0:
                return 0, HALF2, 2, 2
            if k == NS - 1:
                return 128 * (HALF2 + 7 * FULL2), HALF2, 2, 0
            return 128 * (HALF2 + (k - 1) * FULL2), FULL2, 4, 0

        folds = {}
        for k in range(NS):
            off, fw, nkd, kdb = slice_region(k)
            t = slice_pool.tile([128, fw], MM_DT, tag="slice")
            src = pp_d[off : off + 128 * fw].rearrange("(p f) -> p f", f=fw)
            nc.sync.dma_start(t[:], src)
            # whole-slice kw-fold in ONE fully contiguous DVE add: the vp0
            # half plus the vp1 half. Contiguity keeps DVE SBUF-port traffic
            # minimal (strided 8-elem runs waste half of every 32B line and
            # that bank pressure slows DMA/PE under load), and one reader
            # frees the staged tile immediately for the next load.
            F = f_pool.tile([128, nkd * 1024], MM_DT, tag="F")
            nc.vector.tensor_add(
                F[:], t[:, 0 : nkd * 1024], t[:, nkd * 1024 : 2 * nkd * 1024]
            )
            folds[k] = (F, kdb)
            if k == 0:
                continue
            # one PSUM tile (2 banks), one eviction, one store per slice
            # (= 2 output rows): fewer DMAs and semaphores shrink both the
            # serialized scalar work and the fixed end-of-NEFF sem-drain.
            ps = psum_pool.tile([128, 1024], mybir.dt.float32, tag="ps")
            Fa, a_kdb = folds[k]
            Fb, b_kdb = folds[k - 1]
            for rr in range(2):
                # kd-fold: one contiguous DVE add of the two folded planes;
                # output is the j-major matmul rhs directly.
                T = t_pool.tile([128, 1024], MM_DT, tag="T")
                ia, ib = rr - a_kdb, rr + 2 - b_kdb
                nc.vector.tensor_add(
                    T[:],
                    Fa[:, ia * 1024 : (ia + 1) * 1024],
                    Fb[:, ib * 1024 : (ib + 1) * 1024],
                )
                for half in range(2):
                    outseg = ps[:, rr * 512 + half * 256 : rr * 512 + (half + 1) * 256]
                    for j in range(4):
                        # K=128 with zero-padded block-diagonal weights keeps
                        # every matmul at tile_position (0,0): mixing PE tile
                        # positions in one NEFF hangs at runtime.
                        lhsT = wm_sb[:, 512 * half + j * 128 : 512 * half + (j + 1) * 128]
                        rhs = T[:, j * 256 : (j + 1) * 256]
                        nc.tensor.matmul(
                            outseg, lhsT, rhs, start=(j == 0), stop=(j == 3)
                        )
            # evict on ScalarE: evictions wait on matmuls, and in the DVE
            # FIFO they would delay later w-folds, which gate slice loads
            # via slot release. rw's interior 0.5 is folded into wm; the
            # host rescales the 4 global w-edge columns.
            ev = ev_pool.tile([128, 1024], MM_DT, tag="ev")
            nc.scalar.copy(ev[:], ps[:])
            # stores on the scalar ring: a store waiting on eviction must
            # not head-of-line-block the next slice load on the sync ring
            d0 = 2 * (k - 1)
            nc.scalar.dma_start(
                out_d[d0 : d0 + 2].rearrange("d h w c -> h d (w c)"),
                ev[:].rearrange("p (d f) -> p d f", d=2),
            )
    nc.compile()
    return nc


def _host_tables():
    rh = np.where(
        (np.arange(H) < 2) | (np.arange(H) >= H - 2), 1.0, 0.5
    ).astype(np.float32)
    # [half*64+s, whalf*512 + j*128 + h], block-diagonal in (half, whalf).
    # 0.25 = interior rd (0.5) * interior rw (0.5); host rescales d/w edges.
    wm = np.zeros((128, 1024), np.float32)
    s_idx = np.arange(oh)
    for j in range(4):
        h = 2 * s_idx + j
        wm[s_idx, j * 128 + h] = 0.25 * rh[h]
        wm[64 + s_idx, 512 + j * 128 + h] = 0.25 * rh[h]
    return wm.astype(BF16)


def _shard_inputs(patches):
    """Build per-core flat patch blocks. Per slice the layout is
    [p=(uhalf,s)][vpair][kd][j][x=32][t][c] where vpair 0 = kw{0,1} at
    u-slots 1:33 and vpair 1 = kw{2,3} at u-slots 0:32; the two vpair
    halves are contiguous operands of one whole-slice kw-fold add, and
    each folded kd-plane is a contiguous j-major matmul rhs."""
    P5 = np.ascontiguousarray(patches).reshape(B, od, oh, ow, 256).astype(BF16)
    # q-slot k = q+1 for q in [-1, 32); u-slot x = u+1 for u in [-1, 65)
    Pu = np.zeros((B, od + 2, 64, 66, 4, 4, 4, 4), BF16)
    Pu.reshape(B, od + 2, 64, 66, 256)[:, 1 : od + 1, 0:oh, 1 : ow + 1, :] = P5
    pps = []
    for core in range(8):
        b, kc = core // 4, core % 4
        s0 = 8 * kc  # = qbase + 1
        parts = []
        for k in range(NS):
            if k == 0:
                nkd, kdb = 2, 2
            elif k == NS - 1:
                nkd, kdb = 2, 0
            else:
                nkd, kdb = 4, 0
            Q = Pu[b, s0 + k]  # [s=64, u=66, kd, j, v, c]
            blk = np.empty((2, 64, 2, nkd, 4, 32, 2, 4), BF16)
            for uh in range(2):
                for vp in range(2):
                    us = 32 * uh + (1 - vp)
                    # [s, x, kd, j, t, c] -> [s, kd, j, x, t, c]
                    blk[uh, :, vp] = np.transpose(
                        Q[:, us : us + 32, kdb : kdb + nkd, :, 2 * vp : 2 * vp + 2, :],
                        (0, 2, 3, 1, 4, 5),
                    )
            parts.append(blk.reshape(-1))
        pps.append(np.concatenate(parts))
    return pps


def _run(patches, trace=False):
    if "nc" not in _cache:
        _cache["nc"] = _build()
        _cache["tables"] = _host_tables()
    nc = _cache["nc"]
    wm = _cache["tables"]
    pps = _shard_inputs(np.asarray(patches, dtype=np.float32))
    in_maps = [{"pp": pps[core], "wm": wm} for core in range(8)]
    res = bass_utils.run_bass_kernel_spmd(
        nc, in_maps, core_ids=list(range(8)), trace=trace
    )
    out = np.zeros((B, D, H, W, C), np.float32)
    for core in range(8):
        b, kc = core // 4, core % 4
        out[b, RPC * kc : RPC * (kc + 1)] = np.asarray(
            res.results[core]["out"]
        ).astype(np.float32)
    out[:, [0, 1, D - 2, D - 1]] *= 2.0
    out[:, :, :, [0, 1, W - 2, W - 1], :] *= 2.0
    return out, res


def kernel(patches, inputs):
    out, _ = _run(patches)
    return out



# revision 36
# speedup vs baseline: 1.2441x; 1.0450x over previous
"""CombinePatches (3D col2im fold + overlap-count normalize) on 8 TRN2 NeuronCores.

Decomposition (validated numerically against the reference):
  out[b, 2q+kd, 2s+kh, 2u+kw, c] (+)= patches[b, q, s, u, kd, kh, kw, c], then
  out /= cnt, cnt = cd(d)*ch(h)*cw(w) separable overlap counts.

Sharding: 8 cores = B(2) x D-chunks(4). Each core computes 16 output d-rows from
9 od-slices of patches (1 halo slice, zero-padded at global edges by the host).

Per core, per output row d (r=d%2, q=d//2):
  - DVE w-fold: T[s, j, w, c] = A[s, floor(w/2), j, ...] + A[s, floor(w/2)-1, ...]
    done for A = slice q (kd=r) and B = slice q-1 (kd=r+2), with the ow dim
    pre-split into two halves on partitions (p = uhalf*64 + s) so each DVE op
    uses all 128 lanes.
  - TensorE h-fold: O[h, (w,c)] = sum_j Mh_j^T @ T_j accumulated in PSUM over
    (j x {A,B} x {w-half}) = 16 float32r matmuls; 0.25*rh(h) baked into Mh
    (0.25 = interior rd * interior rw).
  - ScalarE eviction: PSUM -> SBUF copy, then DMA store on the scalar ring.
Host fixes the global d-edge rows and w-edge columns by x2 after gather.
"""
import sys

for _p in ("/opt/trn_rl_repo", "/opt/trn_rl_repo/pypackages"):
    if _p not in sys.path:
        sys.path.insert(0, _p)

from contextlib import ExitStack

import numpy as np

import concourse.bass as bass
import concourse.tile as tile
from concourse import bacc, mybir
from concourse import bass_utils

B, D, H, W, C = 2, 64, 128, 128, 4
od, oh, ow = 31, 63, 63
NS = 9              # od-slices per core (incl 1 halo)
RPC = 16            # output d-rows per core
MM_DT = mybir.dt.bfloat16
import ml_dtypes

BF16 = ml_dtypes.bfloat16

# per-partition free width of a slice with nkd kd-planes:
# [vpair=2][kd=nkd][j=4][x=32][t=2][c=4] -- vpair outermost, so the whole
# kw-fold of a slice is ONE fully contiguous DVE add (vp0 half + vp1 half),
# and each folded kd-plane is a contiguous j-major 1024-elem matmul rhs.
def _fw(nkd):
    return 2 * nkd * 1024


# 126 data partitions: p<63 = (uhalf 0, s=p), 63<=p<126 = (uhalf 1, s=p-63).
# The two all-zero s=63 pad rows are never transferred; they would land on
# partitions served by SDMA engine 15, which is ~20% slower than the rest
# and was the straggler that set the load-stream critical path. Matmuls
# run K=126 so the never-written partitions 126/127 are never read.
NP = 126
FULL2, HALF2 = _fw(4), _fw(2)   # DRAM elems/partition per slice
PP_TOTAL = NP * (2 * HALF2 + 7 * FULL2)

_cache = {}


def _build():
    nc = bacc.Bacc(
        "TRN2",
        target_bir_lowering=False,
        debug=False,
        enable_asserts=False,
        num_devices=8,
    )
    # flat pp: [half-slice k=0 (kd 2,3 only)] + [7 full slices] + [half k=8 (kd 0,1)]
    pp_d = nc.dram_tensor(
        "pp", [PP_TOTAL], MM_DT, kind="ExternalInput"
    ).ap()
    wm_d = nc.dram_tensor("wm", [NP, 1024], MM_DT, kind="ExternalInput").ap()
    out_d = nc.dram_tensor(
        "out", [RPC, H, W, C], MM_DT, kind="ExternalOutput"
    ).ap()

    with ExitStack() as ctx:
        tc = ctx.enter_context(tile.TileContext(nc))
        const_pool = ctx.enter_context(tc.tile_pool(name="const", bufs=1))
        # staged slice tiles have exactly one reader (the mega-fold), so
        # slots recycle immediately and a few bufs keep the DMA stream fed
        # without piling up outstanding DMAs (9 outstanding loads measurably
        # degraded early HBM throughput).
        slice_pool = ctx.enter_context(tc.tile_pool(name="slice", bufs=4))
        f_pool = ctx.enter_context(tc.tile_pool(name="fold", bufs=3))
        t_pool = ctx.enter_context(tc.tile_pool(name="tt", bufs=6))
        ev_pool = ctx.enter_context(tc.tile_pool(name="ev", bufs=3))
        psum_pool = ctx.enter_context(tc.tile_pool(name="ps", bufs=3, space="PSUM"))

        # constants go on the scalar-engine HWDGE ring so the sync ring is
        # purely slice loads (HWDGE rings are FIFO per issuing engine).
        wm_sb = const_pool.tile([NP, 1024], MM_DT)
        nc.scalar.dma_start(wm_sb[:], wm_d[:])

        def slice_region(k):
            """(flat offset, free width, n_kd, kd_base) of slice k."""
            if k == 0:
                return 0, HALF2, 2, 2
            if k == NS - 1:
                return NP * (HALF2 + 7 * FULL2), HALF2, 2, 0
            return NP * (HALF2 + (k - 1) * FULL2), FULL2, 4, 0

        folds = {}
        for k in range(NS):
            off, fw, nkd, kdb = slice_region(k)
            t = slice_pool.tile([NP, fw], MM_DT, tag="slice")
            src = pp_d[off : off + NP * fw].rearrange("(p f) -> p f", f=fw)
            nc.sync.dma_start(t[:], src)
            # whole-slice kw-fold in ONE fully contiguous DVE add: the vp0
            # half plus the vp1 half. Contiguity keeps DVE SBUF-port traffic
            # minimal (strided 8-elem runs waste half of every 32B line and
            # that bank pressure slows DMA/PE under load), and one reader
            # frees the staged tile immediately for the next load.
            F = f_pool.tile([NP, nkd * 1024], MM_DT, tag="F")
            nc.vector.tensor_add(
                F[:], t[:, 0 : nkd * 1024], t[:, nkd * 1024 : 2 * nkd * 1024]
            )
            folds[k] = (F, kdb)
            if k == 0:
                continue
            # one PSUM tile (2 banks), one eviction, one store per slice
            # (= 2 output rows): fewer DMAs and semaphores shrink both the
            # serialized scalar work and the fixed end-of-NEFF sem-drain.
            ps = psum_pool.tile([128, 1024], mybir.dt.float32, tag="ps")
            Fa, a_kdb = folds[k]
            Fb, b_kdb = folds[k - 1]
            for rr in range(2):
                # kd-fold: one contiguous DVE add of the two folded planes;
                # output is the j-major matmul rhs directly.
                T = t_pool.tile([NP, 1024], MM_DT, tag="T")
                ia, ib = rr - a_kdb, rr + 2 - b_kdb
                nc.vector.tensor_add(
                    T[:],
                    Fa[:, ia * 1024 : (ia + 1) * 1024],
                    Fb[:, ib * 1024 : (ib + 1) * 1024],
                )
                for half in range(2):
                    outseg = ps[:, rr * 512 + half * 256 : rr * 512 + (half + 1) * 256]
                    for j in range(4):
                        # K=126 (both zero s=63 pad rows dropped from the
                        # transfer); single PE tile position (0,0) as before.
                        lhsT = wm_sb[:, 512 * half + j * 128 : 512 * half + (j + 1) * 128]
                        rhs = T[:, j * 256 : (j + 1) * 256]
                        nc.tensor.matmul(
                            outseg, lhsT, rhs, start=(j == 0), stop=(j == 3)
                        )
            # evict on ScalarE: evictions wait on matmuls, and in the DVE
            # FIFO they would delay later w-folds, which gate slice loads
            # via slot release. rw's interior 0.5 is folded into wm; the
            # host rescales the 4 global w-edge columns.
            ev = ev_pool.tile([128, 1024], MM_DT, tag="ev")
            nc.scalar.copy(ev[:], ps[:])
            # stores on the scalar ring: a store waiting on eviction must
            # not head-of-line-block the next slice load on the sync ring
            d0 = 2 * (k - 1)
            nc.scalar.dma_start(
                out_d[d0 : d0 + 2].rearrange("d h w c -> h d (w c)"),
                ev[:].rearrange("p (d f) -> p d f", d=2),
            )
    nc.compile()
    return nc


def _host_tables():
    rh = np.where(
        (np.arange(H) < 2) | (np.arange(H) >= H - 2), 1.0, 0.5
    ).astype(np.float32)
    # [uhalf*63+s, whalf*512 + j*128 + h], block-diagonal in (uhalf, whalf).
    # 0.25 = interior rd (0.5) * interior rw (0.5); host rescales d/w edges.
    wm = np.zeros((NP, 1024), np.float32)
    s_idx = np.arange(oh)
    for j in range(4):
        h = 2 * s_idx + j
        wm[s_idx, j * 128 + h] = 0.25 * rh[h]
        wm[63 + s_idx, 512 + j * 128 + h] = 0.25 * rh[h]
    return wm.astype(BF16)


def _shard_inputs(patches):
    """Build per-core flat patch blocks. Per slice the layout is
    [p=(uhalf,s)][vpair][kd][j][x=32][t][c] where vpair 0 = kw{0,1} at
    u-slots 1:33 and vpair 1 = kw{2,3} at u-slots 0:32; the two vpair
    halves are contiguous operands of one whole-slice kw-fold add, and
    each folded kd-plane is a contiguous j-major matmul rhs."""
    P5 = np.ascontiguousarray(patches).reshape(B, od, oh, ow, 256).astype(BF16)
    # q-slot k = q+1 for q in [-1, 32); u-slot x = u+1 for u in [-1, 65)
    Pu = np.zeros((B, od + 2, 64, 66, 4, 4, 4, 4), BF16)
    Pu.reshape(B, od + 2, 64, 66, 256)[:, 1 : od + 1, 0:oh, 1 : ow + 1, :] = P5
    pps = []
    for core in range(8):
        b, kc = core // 4, core % 4
        s0 = 8 * kc  # = qbase + 1
        parts = []
        for k in range(NS):
            if k == 0:
                nkd, kdb = 2, 2
            elif k == NS - 1:
                nkd, kdb = 2, 0
            else:
                nkd, kdb = 4, 0
            Q = Pu[b, s0 + k]  # [s=64, u=66, kd, j, v, c]
            blk = np.empty((2, 63, 2, nkd, 4, 32, 2, 4), BF16)
            for uh in range(2):
                for vp in range(2):
                    us = 32 * uh + (1 - vp)
                    # [s, x, kd, j, t, c] -> [s, kd, j, x, t, c]; drop the
                    # all-zero s=63 pad row (partitions are 2*63=126 wide).
                    blk[uh, :, vp] = np.transpose(
                        Q[:63, us : us + 32, kdb : kdb + nkd, :, 2 * vp : 2 * vp + 2, :],
                        (0, 2, 3, 1, 4, 5),
                    )
            parts.append(blk.reshape(-1))
        pps.append(np.concatenate(parts))
    return pps


def _run(patches, trace=False):
    if "nc" not in _cache:
        _cache["nc"] = _build()
        _cache["tables"] = _host_tables()
    nc = _cache["nc"]
    wm = _cache["tables"]
    pps = _shard_inputs(np.asarray(patches, dtype=np.float32))
    in_maps = [{"pp": pps[core], "wm": wm} for core in range(8)]
    res = bass_utils.run_bass_kernel_spmd(
        nc, in_maps, core_ids=list(range(8)), trace=trace
    )
    out = np.zeros((B, D, H, W, C), np.float32)
    for core in range(8):
        b, kc = core // 4, core % 4
        out[b, RPC * kc : RPC * (kc + 1)] = np.asarray(
            res.results[core]["out"]
        ).astype(np.float32)
    out[:, [0, 1, D - 2, D - 1]] *= 2.0
    out[:, :, :, [0, 1, W - 2, W - 1], :] *= 2.0
    return out, res


def kernel(patches, inputs):
    out, _ = _run(patches)
    return out

